# revision 2
# baseline (speedup 1.0000x reference)
"""Trainium2 Bass kernel for 3x3 same-padding Conv2d + bias (NCHW).

Problem: x[16,32,256,256] (*) weight[32,32,3,3] + bias[32] -> out[16,32,256,256]

Strategy (data-parallel over batch, 2 images per NeuronCore on 8 cores):
  - SBUF x layout: partitions = (row-group g, channel ci) = 4*32 = 128;
    image row h lives in row-group (h+1)%4 at "slot" (h+1)//4, each slot is
    258 columns wide (1 zero pad column on each side of the 256 image cols).
    Every input row is DMA'd exactly once.
  - Output computed in "quads" of 4 consecutive rows: PSUM tile [128, 256]
    with partitions = (r, co).  Quad u accumulates 6 matmuls (K=128, M=128,
    N=256): 3 horizontal taps kw for the slot-u window (rows 4u-1..4u+2) and
    3 for the slot-(u+1) window (rows 4u+3..4u+4; other weight rows zero).
  - Weight matrices (6 x [128,128], zero-padded per (g, r_out, kh) validity)
    are precomputed on the host from `weight` and passed as an input.
  - Matmuls run as float32r (full-rate fp32, reduced-precision multiplies).
  - PSUM -> SBUF staging copies alternate VectorE/ScalarE; staging chunks of
    8 quads (32 rows) are DMA'd out as 1 MB transfers.
  - bias is added on the host during the gather (exact for any bias).
"""
import sys

if "/opt/trn_rl_repo" not in sys.path:
    sys.path.insert(0, "/opt/trn_rl_repo")

import numpy as np

B, C, H, W = 16, 32, 256, 256
N_CORES = 8
PER = B // N_CORES          # batches per core
HW = H * W
NSLOT = H // 4 + 1          # 65 row slots
SLOTW = W + 2               # 258 padded columns per slot
NQ = H // 4                 # 64 quads per image

_cache = {}


def _get_nc():
    if "nc" in _cache:
        return _cache["nc"]
    import concourse.mybir as mybir
    import concourse.tile as tile
    import concourse.bass as bass
    from concourse import bacc

    DT = mybir.dt.float32r
    F32 = mybir.dt.float32

    nc = bacc.Bacc("TRN2", target_bir_lowering=False, debug=False,
                   num_devices=N_CORES)
    x_part = nc.dram_tensor("x_part", [PER, C, H, W], DT, kind="ExternalInput")
    w_taps = nc.dram_tensor("w_taps", [6, 128, 128], DT, kind="ExternalInput")
    out_part = nc.dram_tensor("out_part", [PER, C, H, W], F32,
                              kind="ExternalOutput")

    with tile.TileContext(nc) as tc:
        with (
            tc.tile_pool(name="xin", bufs=2) as xpool,
            tc.tile_pool(name="wts", bufs=1) as wpool,
            tc.tile_pool(name="stage", bufs=3) as spool,
            tc.tile_pool(name="psum", bufs=8, space="PSUM") as ppool,
        ):
            w_t = wpool.tile([128, 6, 128], DT)
            nc.sync.dma_start(out=w_t[:],
                              in_=w_taps.ap().rearrange("t k m -> k t m"))

            for b in range(PER):
                xt = xpool.tile([128, NSLOT, SLOTW], DT)
                # zero pads: conv top pad row (-1), bottom pad row (256),
                # junk rows beyond, and the left/right pad columns.
                # (memset rejects float32r; bitcast the view to plain f32)
                nc.vector.memset(xt[0:32, 0, :].bitcast(F32), 0.0)
                nc.vector.memset(xt[32:64, NSLOT - 1, :].bitcast(F32), 0.0)
                nc.vector.memset(xt[64:96, NSLOT - 1, :].bitcast(F32), 0.0)
                nc.vector.memset(xt[96:128, NSLOT - 1, :].bitcast(F32), 0.0)
                nc.vector.memset(xt[:, :, 0].bitcast(F32), 0.0)
                nc.vector.memset(xt[:, :, SLOTW - 1].bitcast(F32), 0.0)

                # row h -> group (h+1)%4, slot (h+1)//4; one DMA per group
                xv = x_part.ap()[b].rearrange("c (s f) w -> c s f w", f=4)
                nc.sync.dma_start(out=xt[0:32, 1:NSLOT, 1:W + 1],
                                  in_=xv[:, :, 3, :])
                nc.sync.dma_start(out=xt[32:64, 0:NSLOT - 1, 1:W + 1],
                                  in_=xv[:, :, 0, :])
                nc.sync.dma_start(out=xt[64:96, 0:NSLOT - 1, 1:W + 1],
                                  in_=xv[:, :, 1, :])
                nc.sync.dma_start(out=xt[96:128, 0:NSLOT - 1, 1:W + 1],
                                  in_=xv[:, :, 2, :])

                for k in range(NQ // 8):          # chunks of 8 quads
                    st = spool.tile([128, 8, W], F32)
                    for ql in range(8):
                        u = k * 8 + ql
                        ps = ppool.tile([128, W], F32)
                        for kw in range(3):
                            nc.tensor.matmul(ps[:], w_t[:, kw * 2, :],
                                             xt[:, u, kw:kw + W],
                                             start=(kw == 0), stop=False)
                            nc.tensor.matmul(ps[:], w_t[:, kw * 2 + 1, :],
                                             xt[:, u + 1, kw:kw + W],
                                             start=False, stop=(kw == 2))
                        if u % 2 == 0:
                            nc.vector.tensor_copy(st[:, ql, :], ps[:])
                        else:
                            nc.scalar.copy(st[:, ql, :], ps[:])
                    base = b * C * HW + k * 32 * W
                    for r in range(4):
                        dst = bass.AP(out_part, base + r * W,
                                      [[HW, 32], [4 * W, 8], [1, W]])
                        nc.sync.dma_start(out=dst, in_=st[r * 32:(r + 1) * 32, :, :])

    nc.compile()
    _cache["nc"] = nc
    return nc


def _make_w_taps(weight):
    """Zero-padded stationary matrices w_taps[kw*2+part][(g,ci), (r,co)]."""
    w_taps = np.zeros((6, 128, 128), dtype=np.float32)
    for kw in range(3):
        for g in range(4):
            for r in range(4):
                kh0 = g - r              # window W_u (input row 4u+g-1)
                if 0 <= kh0 <= 2:
                    w_taps[kw * 2, g * 32:(g + 1) * 32, r * 32:(r + 1) * 32] = \
                        weight[:, :, kh0, kw].T
                kh1 = g - r + 4          # window W_{u+1} (input row 4u+g+3)
                if 0 <= kh1 <= 2:
                    w_taps[kw * 2 + 1, g * 32:(g + 1) * 32, r * 32:(r + 1) * 32] = \
                        weight[:, :, kh1, kw].T
    return w_taps


def kernel(x, weight, bias):
    from concourse.bass_utils import run_bass_kernel_spmd

    x = np.ascontiguousarray(np.asarray(x, dtype=np.float32))
    weight = np.asarray(weight, dtype=np.float32)
    bias = np.asarray(bias, dtype=np.float32)

    nc = _get_nc()
    w_taps = _make_w_taps(weight)
    in_maps = [{"x_part": x[c * PER:(c + 1) * PER], "w_taps": w_taps}
               for c in range(N_CORES)]
    res = run_bass_kernel_spmd(nc, in_maps, list(range(N_CORES)))
    out = np.concatenate([res.results[c]["out_part"] for c in range(N_CORES)],
                         axis=0)
    out += bias.reshape(1, C, 1, 1)
    return out


# revision 3
# speedup vs baseline: 1.5279x; 1.5279x over previous
"""Trainium2 Bass kernel for 3x3 same-padding Conv2d + bias (NCHW).

Problem: x[16,32,256,256] (*) weight[32,32,3,3] + bias[32] -> out[16,32,256,256]

Strategy (data-parallel over batch, 2 images per NeuronCore on 8 cores):
  - Host pre-shuffles x into the SBUF "slot" layout x_shuf[b][(g,ci)][s][258]:
    image row h lives in row-group g=(h+1)%4 at slot s=(h+1)//4; each slot is
    258 wide (zero pad col on each side) so the 3 horizontal conv taps are
    plain free-dim shifts.  Device input DMAs are fully contiguous.
  - Output computed in "quads" of 4 consecutive rows: PSUM tile [128, 256]
    with partitions = (r, co).  Quad u accumulates 6 matmuls (K=128, M=128,
    N=256): 3 horizontal taps kw for the slot-u window (rows 4u-1..4u+2) and
    3 for the slot-(u+1) window (rows 4u+3..4u+4; other weight rows zero).
  - Weight matrices (6 x [128,128], zero-padded per (g, r_out, kh) validity)
    are precomputed on the host from `weight`.
  - Matmuls run as float32r (full-rate fp32, reduced-precision multiplies).
  - PSUM -> SBUF staging copies alternate VectorE/ScalarE; the device writes
    out_shuf[b][(r,co)][q][w] (contiguous per partition) and the host
    unshuffles to NCHW and adds bias (exact for any bias).
"""
import sys

if "/opt/trn_rl_repo" not in sys.path:
    sys.path.insert(0, "/opt/trn_rl_repo")

import numpy as np

B, C, H, W = 16, 32, 256, 256
N_CORES = 8
PER = B // N_CORES          # batches per core
HW = H * W
NSLOT = H // 4 + 1          # 65 row slots
SLOTW = W + 2               # 258 padded columns per slot
NQ = H // 4                 # 64 quads per image
CHUNK = 8                   # quads per staging buffer / out DMA

_cache = {}


def _get_nc():
    if "nc" in _cache:
        return _cache["nc"]
    import concourse.mybir as mybir
    import concourse.tile as tile
    import concourse.bass as bass
    from concourse import bacc

    DT = mybir.dt.float32r
    F32 = mybir.dt.float32

    nc = bacc.Bacc("TRN2", target_bir_lowering=False, debug=False,
                   num_devices=N_CORES)
    x_shuf = nc.dram_tensor("x_shuf", [PER, 128, NSLOT * SLOTW], DT,
                            kind="ExternalInput")
    w_taps = nc.dram_tensor("w_taps", [6, 128, 128], DT, kind="ExternalInput")
    out_shuf = nc.dram_tensor("out_shuf", [PER, 128, NQ * W], F32,
                              kind="ExternalOutput")

    with tile.TileContext(nc) as tc:
        with (
            tc.tile_pool(name="xin", bufs=PER) as xpool,
            tc.tile_pool(name="wts", bufs=1) as wpool,
            tc.tile_pool(name="stage", bufs=3) as spool,
            tc.tile_pool(name="psum", bufs=8, space="PSUM") as ppool,
        ):
            w_t = wpool.tile([128, 6, 128], DT)
            nc.sync.dma_start(out=w_t[:],
                              in_=w_taps.ap().rearrange("t k m -> k t m"))

            # load both batches up front (xpool holds both); each is one
            # fully-contiguous [128, 16770] DMA (~8.6 MB)
            xts = []
            for b in range(PER):
                xt = xpool.tile([128, NSLOT, SLOTW], DT)
                nc.sync.dma_start(
                    out=xt[:],
                    in_=x_shuf.ap()[b].rearrange("p (s w) -> p s w", w=SLOTW))
                xts.append(xt)

            for b in range(PER):
                xt = xts[b]
                for k in range(NQ // CHUNK):
                    st = spool.tile([128, CHUNK, W], F32)
                    for ql in range(CHUNK):
                        u = k * CHUNK + ql
                        ps = ppool.tile([128, W], F32)
                        for kw in range(3):
                            nc.tensor.matmul(ps[:], w_t[:, kw * 2, :],
                                             xt[:, u, kw:kw + W],
                                             start=(kw == 0), stop=False)
                            nc.tensor.matmul(ps[:], w_t[:, kw * 2 + 1, :],
                                             xt[:, u + 1, kw:kw + W],
                                             start=False, stop=(kw == 2))
                        if u % 2 == 0:
                            nc.vector.tensor_copy(st[:, ql, :], ps[:])
                        else:
                            nc.scalar.copy(st[:, ql, :], ps[:])
                    # contiguous per-partition store of CHUNK quads
                    dst = bass.AP(out_shuf, b * 128 * NQ * W + k * CHUNK * W,
                                  [[NQ * W, 128], [1, CHUNK * W]])
                    nc.sync.dma_start(
                        out=dst,
                        in_=st[:].rearrange("p q w -> p (q w)"))

    nc.compile()
    _cache["nc"] = nc
    return nc


def _make_w_taps(weight):
    """Zero-padded stationary matrices w_taps[kw*2+part][(g,ci), (r,co)]."""
    w_taps = np.zeros((6, 128, 128), dtype=np.float32)
    for kw in range(3):
        for g in range(4):
            for r in range(4):
                kh0 = g - r              # window W_u (input row 4u+g-1)
                if 0 <= kh0 <= 2:
                    w_taps[kw * 2, g * 32:(g + 1) * 32, r * 32:(r + 1) * 32] = \
                        weight[:, :, kh0, kw].T
                kh1 = g - r + 4          # window W_{u+1} (input row 4u+g+3)
                if 0 <= kh1 <= 2:
                    w_taps[kw * 2 + 1, g * 32:(g + 1) * 32, r * 32:(r + 1) * 32] = \
                        weight[:, :, kh1, kw].T
    return w_taps


def _shuffle_x(x):
    """x[B,C,H,W] -> x_shuf[B,128,NSLOT,SLOTW]: row h -> (group (h+1)%4,
    slot (h+1)//4), cols 1..W, zero pads elsewhere."""
    xs = np.zeros((B, 128, NSLOT, SLOTW), dtype=np.float32)
    # group g, slot s holds row 4s+g-1
    xs[:, 0:32, 1:NSLOT, 1:W + 1] = x[:, :, 3::4, :].transpose(0, 1, 2, 3)
    xs[:, 32:64, 0:NSLOT - 1, 1:W + 1] = x[:, :, 0::4, :]
    xs[:, 64:96, 0:NSLOT - 1, 1:W + 1] = x[:, :, 1::4, :]
    xs[:, 96:128, 0:NSLOT - 1, 1:W + 1] = x[:, :, 2::4, :]
    return xs.reshape(B, 128, NSLOT * SLOTW)


def _unshuffle_out(chunks):
    """chunks: list of PER-core arrays [PER,128,NQ*W] -> out[B,C,H,W]."""
    o = np.concatenate(chunks, axis=0)              # [B, 128, NQ*W]
    o = o.reshape(B, 4, C, NQ, W)                   # [(r c), q, w]
    o = o.transpose(0, 2, 3, 1, 4)                  # [B, C, q, r, w]
    return np.ascontiguousarray(o.reshape(B, C, H, W))


def kernel(x, weight, bias):
    from concourse.bass_utils import run_bass_kernel_spmd

    x = np.asarray(x, dtype=np.float32)
    weight = np.asarray(weight, dtype=np.float32)
    bias = np.asarray(bias, dtype=np.float32)

    nc = _get_nc()
    w_taps = _make_w_taps(weight)
    x_shuf = _shuffle_x(x)
    in_maps = [{"x_shuf": x_shuf[c * PER:(c + 1) * PER], "w_taps": w_taps}
               for c in range(N_CORES)]
    res = run_bass_kernel_spmd(nc, in_maps, list(range(N_CORES)))
    out = _unshuffle_out([res.results[c]["out_shuf"] for c in range(N_CORES)])
    out += bias.reshape(1, C, 1, 1)
    return out
